# revision 20
# baseline (speedup 1.0000x reference)
"""Trainium2 Bass kernel for nn_Attention_14190571946482.

Single-head causal self-attention with the TF band_part mask quirk:
  q = x @ Wq ; k = x @ Wv ; v = x @ Wk        (naming quirk of the original)
  score = q @ k^T / sqrt(D) + NEG*(j >= i)    (diagonal masked too!)
  out   = softmax(score) @ v
Row 0 is fully masked -> its softmax is exactly uniform over all S
positions, i.e. out[0] = mean_s v[s]; computed via a mean-of-x matmul.

Sharding: 8 cores = 4 batches x 2 roles. Role r owns q-tiles {2i+r}
(128 rows each). Slot i of the SPMD program processes 2(i+1) causal
kk-chunks of 128; role 0's tile 2i needs 2i+1 chunks (one wasted, its
mask is all-zero), role 1's tile 2i+1 needs exactly 2i+2. All
role-dependent structure (qx column gather, chunk masks, row-0 blend
selectors) is carried in input data; the program is identical (SPMD).

Score is computed TRANSPOSED, scoreT[kk, q] = sum_u kT[u,kk]*qT[u,q],
so exp(scoreT) in SBUF is directly the lhsT of the context matmul
(ctx[q,u] = sum_kk attnT[kk,q]*v[kk,u]) -- no PE transposes. Row sums
come from an N=1 matmul with a ones vector accumulated per slot.
Projections (kT g-blocks, v chunks) are interleaved with attention
slots in xT-chunk DMA arrival order so the PE streams without gaps.
"""

import sys

sys.path.insert(0, "/opt/trn_rl_repo")

import numpy as np
import ml_dtypes

import concourse.bass as bass
import concourse.bacc as bacc
import concourse.mybir as mybir
from concourse.tile import TileContext
from concourse import bass_utils

BF16 = ml_dtypes.bfloat16

B, S, D, U = 4, 2048, 512, 512
P = 128
NSLOT = 8
SCALE = 1.0 / float(np.sqrt(np.float32(D)))

_nc_cache = None


def build_nc():
    global _nc_cache
    if _nc_cache is not None:
        return _nc_cache

    f32 = mybir.dt.float32
    bf16 = mybir.dt.bfloat16

    nc = bacc.Bacc()
    xT_d = nc.declare_dram_parameter("xT", [D, S], bf16, isOutput=False)
    qx_d = nc.declare_dram_parameter("qx", [D, NSLOT * P], bf16, isOutput=False)
    wq_d = nc.declare_dram_parameter("wq", [D, U], bf16, isOutput=False)
    wv_d = nc.declare_dram_parameter("wv", [D, U], bf16, isOutput=False)
    wk_d = nc.declare_dram_parameter("wk", [D, U], bf16, isOutput=False)
    mk_d = nc.declare_dram_parameter("msk", [P, 2 * P], bf16, isOutput=False)
    rs_d = nc.declare_dram_parameter("rsel", [1, 2], f32, isOutput=False)
    se_d = nc.declare_dram_parameter("sume", [P, 1], f32, isOutput=False)
    out_d = nc.declare_dram_parameter("out", [NSLOT * P, U], bf16, isOutput=True)

    with TileContext(nc) as tc:
        with (
            tc.tile_pool(name="cst", bufs=1) as cst,
            tc.tile_pool(name="att", bufs=6) as att,
            tc.tile_pool(name="work", bufs=4) as work,
            tc.tile_pool(name="outp", bufs=2) as outp,
            tc.tile_pool(name="small", bufs=8) as small,
            tc.tile_pool(name="psA", bufs=2, space="PSUM") as psA,
            tc.tile_pool(name="psS", bufs=1, space="PSUM") as psS,
            tc.tile_pool(name="psC", bufs=2, space="PSUM") as psC,
            tc.tile_pool(name="psR", bufs=1, space="PSUM") as psR,
        ):
            # ---- input DMA issues (sync/gpsimd/scalar queues; data needed
            # earliest first: wq+qx unblock qT, then wv+xT0 for kT g0) ----
            wq_t = cst.tile([P, 4, U], bf16, tag="wq")
            wq_r = wq_d.rearrange("(d p) u -> p d u", p=P)
            qx_t = cst.tile([P, 4, NSLOT * P], bf16, tag="qx")
            qx_r = qx_d.rearrange("(d p) s -> p d s", p=P)
            # Each issuing engine owns one serial DMA descriptor ring; rings
            # share HBM bandwidth. So: per-ring issue order = arrival
            # priority, and early-needed pieces are kept small.
            # constants first on gpsimd (warmup matmuls depend on wu)
            wu = cst.tile([P, 512], bf16, tag="wu")
            nc.gpsimd.memset(wu, 0.0)
            ones = cst.tile([P, 1], bf16, tag="ones")
            nc.gpsimd.memset(ones, 1.0)

            nc.sync.dma_start(out=wq_t[:, :, 0:P], in_=wq_r[:, :, 0:P])
            nc.sync.dma_start(out=qx_t[:, :, 0:256], in_=qx_r[:, :, 0:256])
            nc.sync.dma_start(out=qx_t[:, :, 256:512], in_=qx_r[:, :, 256:512])
            nc.sync.dma_start(out=wq_t[:, :, P:U], in_=wq_r[:, :, P:U])
            nc.sync.dma_start(out=qx_t[:, :, 512:1024], in_=qx_r[:, :, 512:1024])

            xT_t = cst.tile([P, 4, S], bf16, tag="xT")
            xT_r = xT_d.rearrange("(d p) s -> p d s", p=P)
            for g in range(4):
                nc.scalar.dma_start(out=xT_t[:, :, g * 512:(g + 1) * 512],
                                    in_=xT_r[:, :, g * 512:(g + 1) * 512])

            msk_t = cst.tile([P, 2 * P], bf16, tag="msk")
            nc.gpsimd.dma_start(out=msk_t, in_=mk_d[:, :])
            wv_t = cst.tile([P, 4, U], bf16, tag="wv")
            nc.gpsimd.dma_start(out=wv_t, in_=wv_d.rearrange("(d p) u -> p d u", p=P))
            wk_t = cst.tile([P, 4, U], bf16, tag="wk")
            nc.gpsimd.dma_start(out=wk_t, in_=wk_d.rearrange("(d p) u -> p d u", p=P))
            rsel = cst.tile([1, 2], f32, tag="rsel")
            nc.gpsimd.dma_start(out=rsel, in_=rs_d[:, :])
            sume = cst.tile([P, 1], f32, tag="sume")
            nc.gpsimd.dma_start(out=sume, in_=se_d[:, :])

            wq = [wq_t[:, d, :] for d in range(4)]
            qx = [qx_t[:, d, :] for d in range(4)]
            xT = [xT_t[:, d, :] for d in range(4)]
            wv = [wv_t[:, d, :] for d in range(4)]
            wk = [wk_t[:, d, :] for d in range(4)]
            mask0 = msk_t[:, 0:P]
            mask1 = msk_t[:, P:2 * P]

            # ---- PE warm-up / fillers: dummy matmuls keep the PE busy (and
            # the HAM clock ramping) while input DMAs are still in flight.
            # Any PE idle gap resets the ramp, costing ~2x on what follows.
            # Fillers must be dependency-free: during the ramp psC is not yet
            # used by any slot, so its pool rotation creates no cross-engine
            # WARs (psA would chain fillers onto the DMA-gated qT copies).
            def filler(n, pool):
                for _ in range(n):
                    wups = pool.tile([P, 512], f32,
                                     tag="blk" if pool is psA else "ctx")
                    nc.tensor.matmul(wups[:, 0:256], lhsT=wu[:, :P],
                                     rhs=wu[:, 0:256], start=True, stop=True)

            filler(28, psA)

            # ---- persistent PSUM tiles for scoreT chunks: 3 banks x 4
            # quarters, chunk gc -> bank gc%3, quarter (gc//3)%4. With the
            # lead-2 pipeline (ctx_c emitted after sc_{c+2}) the PE never
            # writes a bank while scalar exp-reads another quarter of it:
            # exp_c overlaps only sc_{c+1}/sc_{c+2} (different banks), and
            # sc_{c+3} (same bank) queues after ctx_c which waits on exp_c.
            # (PE-write + engine-read of one PSUM bank is fatal on HW.)
            scT = [psS.tile([P, 512], f32, tag=f"sc{i}", name=f"sc{i}")
                   for i in range(3)]
            rsum = psR.tile([P, 1], f32, tag="rsum")

            # ---- qT [u, 8*128] from gathered qx ----
            qT = [cst.tile([P, NSLOT * P], bf16, tag=f"qT{u}", name=f"qT{u}")
                  for u in range(4)]
            def emit_qt(u, h):
                ps = psA.tile([P, 512], f32, tag="blk")
                for d in range(4):
                    nc.tensor.matmul(
                        ps,
                        lhsT=wq[d][:, u * P:(u + 1) * P],
                        rhs=qx[d][:, h * 512:(h + 1) * 512],
                        start=(d == 0), stop=(d == 3),
                    )
                nc.scalar.copy(qT[u][:, h * 512:(h + 1) * 512], ps)

            emit_qt(0, 0)
            filler(6, psC)  # cover the wq-tail / xTg0 arrival stagger

            kT = [cst.tile([P, S], bf16, tag=f"kT{u}", name=f"kT{u}")
                  for u in range(4)]
            v_sb = [cst.tile([P, U], bf16, tag=f"v{sc}", name=f"v{sc}")
                    for sc in range(16)]

            gc = 0  # global chunk counter for psS rotation

            def emit_score(slot, c):
                """scoreT chunk c of slot: [128kk, 128q] psum quarter."""
                nonlocal gc
                t_ps = scT[gc % 3]
                q4 = (gc // 3) % 4
                ps = t_ps[:, q4 * P:(q4 + 1) * P]
                gc += 1
                for d in range(4):
                    nc.tensor.matmul(
                        ps,
                        lhsT=kT[d][:, c * P:(c + 1) * P],
                        rhs=qT[d][:, slot * P:(slot + 1) * P],
                        start=(d == 0), stop=(d == 3),
                    )
                return ps

            def emit_exp(ps, n, c):
                """exp (scalar) + mask multiply (vector) for the last two."""
                a = att.tile([P, P], bf16, tag="attnT")
                if c >= n - 2:
                    raw = work.tile([P, P], bf16, tag="raw")
                    nc.scalar.activation(raw, ps,
                                         mybir.ActivationFunctionType.Exp,
                                         scale=SCALE)
                    nc.vector.tensor_mul(a, raw, mask0 if c == n - 2 else mask1)
                else:
                    nc.scalar.activation(a, ps,
                                         mybir.ActivationFunctionType.Exp,
                                         scale=SCALE)
                return a

            def emit_ctx(ctx_ps, a, c, n):
                nc.tensor.matmul(ctx_ps, lhsT=a, rhs=v_sb[c],
                                 start=(c == 0), stop=(c == n - 1))
                nc.tensor.matmul(rsum, lhsT=a, rhs=ones,
                                 start=(c == 0), stop=(c == n - 1))

            ctx0_sb = cst.tile([P, 512], bf16, tag="ctx0")

            def emit_slot(i):
                """Attention for slot i: n = 2(i+1) chunks, 1-deep pipeline."""
                n = 2 * (i + 1)
                ctx_ps = psC.tile([P, 512], f32, tag="ctx")
                pending = []  # (attnT tile, chunk)
                for c in range(n):
                    ps = emit_score(i, c)
                    a = emit_exp(ps, n, c)
                    if len(pending) >= 2:
                        pa, pc = pending.pop(0)
                        emit_ctx(ctx_ps, pa, pc, n)
                    pending.append((a, c))
                while pending:
                    pa, pc = pending.pop(0)
                    emit_ctx(ctx_ps, pa, pc, n)
                # epilogue entirely on vector so scalar stays free for exps
                rcp = small.tile([P, 1], f32, tag="rcp")
                if i == 0:
                    rs2 = small.tile([P, 1], f32, tag="rs2")
                    nc.vector.tensor_add(rs2, rsum, sume)
                    nc.vector.reciprocal(rcp, rs2)
                else:
                    nc.vector.reciprocal(rcp, rsum)
                if i == 0:
                    nc.vector.tensor_scalar_mul(ctx0_sb, ctx_ps, rcp)
                    # blend + store deferred until mean-v is ready
                else:
                    ctx16 = outp.tile([P, 512], bf16, tag="ctxs")
                    nc.vector.tensor_scalar_mul(ctx16, ctx_ps, rcp)
                    nc.sync.dma_start(out=out_d[i * P:(i + 1) * P, :], in_=ctx16)

            def emit_ktg(g):
                for u in range(4):
                    ps = psA.tile([P, 512], f32, tag="blk")
                    for d in range(4):
                        nc.tensor.matmul(
                            ps,
                            lhsT=wv[d][:, u * P:(u + 1) * P],
                            rhs=xT[d][:, g * 512:(g + 1) * 512],
                            start=(d == 0), stop=(d == 3),
                        )
                    if u % 2 == 0:
                        nc.scalar.copy(kT[u][:, g * 512:(g + 1) * 512], ps)
                    else:
                        nc.vector.tensor_copy(kT[u][:, g * 512:(g + 1) * 512], ps)

            def emit_vg(g):
                for sc in range(4 * g, 4 * g + 4):
                    ps = psA.tile([P, 512], f32, tag="blk")
                    for d in range(4):
                        nc.tensor.matmul(
                            ps,
                            lhsT=xT[d][:, sc * P:(sc + 1) * P],
                            rhs=wk[d],
                            start=(d == 0), stop=(d == 3),
                        )
                    if sc % 2 == 0:
                        nc.scalar.copy(v_sb[sc], ps)
                    else:
                        nc.vector.tensor_copy(v_sb[sc], ps)

            # ---- streaming schedule: projections per xT chunk, attention
            # slots as their kT/v dependencies complete ----
            emit_ktg(0)
            for u in (1, 2, 3):
                emit_qt(u, 0)  # wq tail arrives during kTg0
            emit_vg(0)
            emit_slot(0)
            emit_slot(1)
            emit_ktg(1)
            emit_vg(1)
            for u in range(4):
                emit_qt(u, 1)  # only needed from slot 4 on; qx1 arrives late
            emit_slot(2)
            emit_slot(3)
            emit_ktg(2)
            emit_vg(2)
            emit_slot(4)
            emit_slot(5)
            emit_ktg(3)
            emit_vg(3)

            # ---- mean-of-v (for the fully-masked global row 0) ----
            xs16 = []
            for d in range(4):
                xs = small.tile([P, 1], f32, tag="xs")
                nc.vector.reduce_sum(xs, xT[d], axis=mybir.AxisListType.X)
                x16 = small.tile([P, 1], bf16, tag="xs16")
                nc.vector.tensor_copy(x16, xs)
                xs16.append(x16)
            vm_ps = psA.tile([1, 512], f32, tag="blk")
            for d in range(4):
                nc.tensor.matmul(vm_ps, lhsT=xs16[d], rhs=wk[d],
                                 start=(d == 0), stop=(d == 3))
            vm_sb = cst.tile([1, 512], f32, tag="vm_sb")
            # vm_sb = sum_s v[s, :] * rscale  (rscale = 1/S for role 0, else 0)
            nc.vector.tensor_scalar_mul(vm_sb, vm_ps, rsel[0:1, 1:2])
            # row 0 of role 0 = mean(v): ctx*rsel + sum(v)*rscale
            nc.vector.tensor_scalar_mul(ctx0_sb[0:1, :], ctx0_sb[0:1, :],
                                        rsel[0:1, 0:1])
            nc.vector.tensor_add(ctx0_sb[0:1, :], ctx0_sb[0:1, :], vm_sb)
            nc.sync.dma_start(out=out_d[0:P, :], in_=ctx0_sb)

            emit_slot(6)
            emit_slot(7)

    nc.compile()
    _nc_cache = nc
    return nc


def host_inputs(query, Wq, Wv, Wk):
    """Build per-core input maps. query [B,S,D] f32; W* [D,U] f32."""
    wq16 = Wq.astype(BF16)
    wv16 = Wv.astype(BF16)
    wk16 = Wk.astype(BF16)

    i = np.arange(P)[:, None]
    j = np.arange(P)[None, :]
    tri = (i < j).astype(np.float32)  # keep kk < q within the diagonal chunk
    masks = {
        0: np.concatenate([tri, np.zeros((P, P), np.float32)], 1).astype(BF16),
        1: np.concatenate([np.ones((P, P), np.float32), tri], 1).astype(BF16),
    }

    in_maps = []
    for core in range(8):
        b, r = core // 2, core % 2
        xTb = np.ascontiguousarray(query[b].T).astype(BF16)       # [D, S]
        cols = np.concatenate(
            [np.arange(P * (2 * s + r), P * (2 * s + r) + P)
             for s in range(NSLOT)]
        )
        qx = np.ascontiguousarray(xTb[:, cols])                    # [D, 1024]
        rsel = np.array([[0.0, 1.0 / S]] if r == 0 else [[1.0, 0.0]], np.float32)
        sume = np.zeros((P, 1), np.float32)
        if r == 0:
            sume[0, 0] = 1.0  # avoid 1/0 on the fully-masked row
        in_maps.append({
            "xT": xTb, "qx": qx,
            "wq": wq16, "wv": wv16, "wk": wk16,
            "msk": masks[r], "rsel": rsel, "sume": sume,
        })
    return in_maps


def assemble_output(results):
    """results: list of 8 dicts with 'out' [1024, 512] bf16."""
    out = np.zeros((B, S, U), np.float32)
    for core in range(8):
        b, r = core // 2, core % 2
        o = np.asarray(results[core]["out"]).astype(np.float32)
        for s in range(NSLOT):
            t = 2 * s + r
            out[b, P * t:P * (t + 1), :] = o[P * s:P * (s + 1), :]
    return out


def run(query, Wq, Wv, Wk, **kwargs):
    """Build, compile, and execute on all 8 cores. Returns (output, results)."""
    nc = build_nc()
    in_maps = host_inputs(
        np.asarray(query, np.float32), np.asarray(Wq, np.float32),
        np.asarray(Wv, np.float32), np.asarray(Wk, np.float32),
    )
    res = bass_utils.run_bass_kernel_spmd(nc, in_maps, list(range(8)), **kwargs)
    return assemble_output(res.results), res


def kernel(query, Wq, Wv, Wk):
    out, _ = run(query, Wq, Wv, Wk)
    return out


if __name__ == "__main__":
    rng = np.random.default_rng(0)
    q = rng.standard_normal((B, S, D), dtype=np.float32)
    scale = np.sqrt(2.0 / (D + U)).astype(np.float32)
    Wq = rng.standard_normal((D, U), dtype=np.float32) * scale
    Wv = rng.standard_normal((D, U), dtype=np.float32) * scale
    Wk = rng.standard_normal((D, U), dtype=np.float32) * scale
    out = kernel(q, Wq, Wv, Wk)
    print(out.shape, out.dtype, np.abs(out).mean())


# revision 25
# speedup vs baseline: 1.0207x; 1.0207x over previous
"""Trainium2 Bass kernel for nn_Attention_14190571946482.

Single-head causal self-attention with the TF band_part mask quirk:
  q = x @ Wq ; k = x @ Wv ; v = x @ Wk        (naming quirk of the original)
  score = q @ k^T / sqrt(D) + NEG*(j >= i)    (diagonal masked too!)
  out   = softmax(score) @ v
Row 0 is fully masked -> its softmax is exactly uniform over all S
positions, i.e. out[0] = mean_s v[s]; computed via a mean-of-x matmul.

Sharding: 8 cores = 4 batches x 2 roles. Role r owns q-tiles {2i+r}
(128 rows each). Slot i of the SPMD program processes 2(i+1) causal
kk-chunks of 128; role 0's tile 2i needs 2i+1 chunks (one wasted, its
mask is all-zero), role 1's tile 2i+1 needs exactly 2i+2. All
role-dependent structure (qx column gather, chunk masks, row-0 blend
selectors) is carried in input data; the program is identical (SPMD).

Score is computed TRANSPOSED, scoreT[kk, q] = sum_u kT[u,kk]*qT[u,q],
so exp(scoreT) in SBUF is directly the lhsT of the context matmul
(ctx[q,u] = sum_kk attnT[kk,q]*v[kk,u]) -- no PE transposes. Row sums
come from an N=1 matmul with a ones vector accumulated per slot.
Projections (kT g-blocks, v chunks) are interleaved with attention
slots in xT-chunk DMA arrival order so the PE streams without gaps.
"""

import sys

sys.path.insert(0, "/opt/trn_rl_repo")

import numpy as np
import ml_dtypes

import concourse.bass as bass
import concourse.bacc as bacc
import concourse.mybir as mybir
from concourse.tile import TileContext
from concourse import bass_utils

BF16 = ml_dtypes.bfloat16

B, S, D, U = 4, 2048, 512, 512
P = 128
NSLOT = 8
SCALE = 1.0 / float(np.sqrt(np.float32(D)))

_nc_cache = None


def build_nc():
    global _nc_cache
    if _nc_cache is not None:
        return _nc_cache

    f32 = mybir.dt.float32
    bf16 = mybir.dt.bfloat16

    nc = bacc.Bacc()
    xT_d = nc.declare_dram_parameter("xT", [D, S], bf16, isOutput=False)
    qx_d = nc.declare_dram_parameter("qx", [D, NSLOT * P], bf16, isOutput=False)
    wq_d = nc.declare_dram_parameter("wq", [D, U], bf16, isOutput=False)
    wv_d = nc.declare_dram_parameter("wv", [D, U], bf16, isOutput=False)
    wk_d = nc.declare_dram_parameter("wk", [D, U], bf16, isOutput=False)
    mk_d = nc.declare_dram_parameter("msk", [P, 2 * P], bf16, isOutput=False)
    rs_d = nc.declare_dram_parameter("rsel", [1, 2], f32, isOutput=False)
    se_d = nc.declare_dram_parameter("sume", [P, 1], f32, isOutput=False)
    out_d = nc.declare_dram_parameter("out", [NSLOT * P, U], bf16, isOutput=True)

    with TileContext(nc) as tc:
        with (
            tc.tile_pool(name="cst", bufs=1) as cst,
            tc.tile_pool(name="att", bufs=6) as att,
            tc.tile_pool(name="work", bufs=4) as work,
            tc.tile_pool(name="outp", bufs=2) as outp,
            tc.tile_pool(name="small", bufs=8) as small,
            tc.tile_pool(name="psA", bufs=2, space="PSUM") as psA,
            tc.tile_pool(name="psS", bufs=1, space="PSUM") as psS,
            tc.tile_pool(name="psC", bufs=2, space="PSUM") as psC,
            tc.tile_pool(name="psR", bufs=1, space="PSUM") as psR,
        ):
            # ---- input DMA issues (sync/gpsimd/scalar queues; data needed
            # earliest first: wq+qx unblock qT, then wv+xT0 for kT g0) ----
            # Each issuing engine owns one serial DMA descriptor ring; rings
            # share HBM bandwidth, so per-ring issue order = arrival priority.
            # DMA-write dependencies are tracked at TILE granularity: every
            # chunk that must unblock compute separately gets its OWN tile.
            # constants first on gpsimd (warmup matmuls depend on wu)
            wu = cst.tile([P, 512], bf16, tag="wu")
            nc.vector.memset(wu, 0.0)
            ones = cst.tile([P, 1], bf16, tag="ones")
            nc.vector.memset(ones, 1.0)

            wq_r = wq_d.rearrange("(d p) u -> p d u", p=P)
            qx_r = qx_d.rearrange("(d p) s -> p d s", p=P)
            wq0_t = cst.tile([P, 4, P], bf16, tag="wq0")
            nc.sync.dma_start(out=wq0_t, in_=wq_r[:, :, 0:P])
            qx0_t = cst.tile([P, 4, 512], bf16, tag="qx0")
            nc.sync.dma_start(out=qx0_t, in_=qx_r[:, :, 0:512])
            wq1_t = cst.tile([P, 4, U - P], bf16, tag="wq1")
            nc.sync.dma_start(out=wq1_t, in_=wq_r[:, :, P:U])
            qx1_t = cst.tile([P, 4, 512], bf16, tag="qx1")
            nc.sync.dma_start(out=qx1_t, in_=qx_r[:, :, 512:1024])

            xT_r = xT_d.rearrange("(d p) s -> p d s", p=P)
            xTg_t = []
            for g in range(4):
                t = cst.tile([P, 4, 512], bf16, tag=f"xTg{g}", name=f"xTg{g}")
                nc.scalar.dma_start(out=t, in_=xT_r[:, :, g * 512:(g + 1) * 512])
                xTg_t.append(t)

            wv_t = cst.tile([P, 4, U], bf16, tag="wv")
            nc.gpsimd.dma_start(out=wv_t, in_=wv_d.rearrange("(d p) u -> p d u", p=P))
            wk_t = cst.tile([P, 4, U], bf16, tag="wk")
            nc.gpsimd.dma_start(out=wk_t, in_=wk_d.rearrange("(d p) u -> p d u", p=P))
            msk_t = cst.tile([P, 2 * P], bf16, tag="msk")
            nc.gpsimd.dma_start(out=msk_t, in_=mk_d[:, :])
            rsel = cst.tile([1, 2], f32, tag="rsel")
            nc.gpsimd.dma_start(out=rsel, in_=rs_d[:, :])
            sume = cst.tile([P, 1], f32, tag="sume")
            nc.gpsimd.dma_start(out=sume, in_=se_d[:, :])

            def wq(d, lo, hi):  # wq columns [lo:hi] of d-slab, split tiles
                if hi <= P:
                    return wq0_t[:, d, lo:hi]
                return wq1_t[:, d, lo - P:hi - P]

            qx = [[qx0_t[:, d, :] for d in range(4)],
                  [qx1_t[:, d, :] for d in range(4)]]
            xTg = [[xTg_t[g][:, d, :] for d in range(4)] for g in range(4)]
            wv = [wv_t[:, d, :] for d in range(4)]
            wk = [wk_t[:, d, :] for d in range(4)]
            mask0 = msk_t[:, 0:P]
            mask1 = msk_t[:, P:2 * P]

            # ---- PE warm-up / fillers: dummy matmuls keep the PE busy (and
            # the HAM clock ramping) while input DMAs are still in flight.
            # Any PE idle gap resets the ramp, costing ~2x on what follows.
            # Fillers must be dependency-free: during the ramp psC is not yet
            # used by any slot, so its pool rotation creates no cross-engine
            # WARs (psA would chain fillers onto the DMA-gated qT copies).
            def filler(n, pool):
                for _ in range(n):
                    wups = pool.tile([P, 512], f32,
                                     tag="blk" if pool is psA else "ctx")
                    nc.tensor.matmul(wups[:, 0:256], lhsT=wu[:, :P],
                                     rhs=wu[:, 0:256], start=True, stop=True)

            filler(28, psA)

            # ---- persistent PSUM tiles for scoreT chunks: 3 banks x 4
            # quarters, chunk gc -> bank gc%3, quarter (gc//3)%4. With the
            # lead-2 pipeline (ctx_c emitted after sc_{c+2}) the PE never
            # writes a bank while scalar exp-reads another quarter of it:
            # exp_c overlaps only sc_{c+1}/sc_{c+2} (different banks), and
            # sc_{c+3} (same bank) queues after ctx_c which waits on exp_c.
            # (PE-write + engine-read of one PSUM bank is fatal on HW.)
            scT = [psS.tile([P, 512], f32, tag=f"sc{i}", name=f"sc{i}")
                   for i in range(3)]
            rsum = psR.tile([P, 1], f32, tag="rsum")

            # ---- qT [u, 8*128] from gathered qx ----
            qT = [cst.tile([P, NSLOT * P], bf16, tag=f"qT{u}", name=f"qT{u}")
                  for u in range(4)]
            def emit_qt(u, h):
                ps = psA.tile([P, 512], f32, tag="blk")
                for d in range(4):
                    nc.tensor.matmul(
                        ps,
                        lhsT=wq(d, u * P, (u + 1) * P),
                        rhs=qx[h][d],
                        start=(d == 0), stop=(d == 3),
                    )
                nc.scalar.copy(qT[u][:, h * 512:(h + 1) * 512], ps)

            emit_qt(0, 0)
            filler(6, psC)  # cover the wq-tail / xTg0 arrival stagger

            kT = [cst.tile([P, S], bf16, tag=f"kT{u}", name=f"kT{u}")
                  for u in range(4)]
            v_sb = [cst.tile([P, U], bf16, tag=f"v{sc}", name=f"v{sc}")
                    for sc in range(16)]

            gc = 0  # global chunk counter for psS rotation

            def emit_score(slot, c):
                """scoreT chunk c of slot: [128kk, 128q] psum quarter."""
                nonlocal gc
                t_ps = scT[gc % 3]
                q4 = (gc // 3) % 4
                ps = t_ps[:, q4 * P:(q4 + 1) * P]
                gc += 1
                for d in range(4):
                    nc.tensor.matmul(
                        ps,
                        lhsT=kT[d][:, c * P:(c + 1) * P],
                        rhs=qT[d][:, slot * P:(slot + 1) * P],
                        start=(d == 0), stop=(d == 3),
                    )
                return ps

            def emit_exp(ps, n, c):
                """exp (scalar) + mask multiply (vector) for the last two."""
                a = att.tile([P, P], bf16, tag="attnT")
                if c >= n - 2:
                    raw = work.tile([P, P], bf16, tag="raw")
                    nc.scalar.activation(raw, ps,
                                         mybir.ActivationFunctionType.Exp,
                                         scale=SCALE)
                    nc.vector.tensor_mul(a, raw, mask0 if c == n - 2 else mask1)
                else:
                    nc.scalar.activation(a, ps,
                                         mybir.ActivationFunctionType.Exp,
                                         scale=SCALE)
                return a

            def emit_ctx(ctx_ps, a, c, n):
                nc.tensor.matmul(ctx_ps, lhsT=a, rhs=v_sb[c],
                                 start=(c == 0), stop=(c == n - 1))
                nc.tensor.matmul(rsum, lhsT=a, rhs=ones,
                                 start=(c == 0), stop=(c == n - 1))

            ctx0_sb = cst.tile([P, 512], bf16, tag="ctx0")

            def emit_slot(i):
                """Attention for slot i: n = 2(i+1) chunks, 1-deep pipeline."""
                n = 2 * (i + 1)
                ctx_ps = psC.tile([P, 512], f32, tag="ctx")
                pending = []  # (attnT tile, chunk)
                for c in range(n):
                    ps = emit_score(i, c)
                    a = emit_exp(ps, n, c)
                    if len(pending) >= 2:
                        pa, pc = pending.pop(0)
                        emit_ctx(ctx_ps, pa, pc, n)
                    pending.append((a, c))
                while pending:
                    pa, pc = pending.pop(0)
                    emit_ctx(ctx_ps, pa, pc, n)
                # epilogue entirely on vector so scalar stays free for exps
                rcp = small.tile([P, 1], f32, tag="rcp")
                if i == 0:
                    rs2 = small.tile([P, 1], f32, tag="rs2")
                    nc.vector.tensor_add(rs2, rsum, sume)
                    nc.vector.reciprocal(rcp, rs2)
                else:
                    nc.vector.reciprocal(rcp, rsum)
                if i == 0:
                    nc.vector.tensor_scalar_mul(ctx0_sb, ctx_ps, rcp)
                    # blend + store deferred until mean-v is ready
                else:
                    ctx16 = outp.tile([P, 512], bf16, tag="ctxs")
                    nc.vector.tensor_scalar_mul(ctx16, ctx_ps, rcp)
                    nc.sync.dma_start(out=out_d[i * P:(i + 1) * P, :], in_=ctx16)

            def emit_ktg(g):
                for u in range(4):
                    ps = psA.tile([P, 512], f32, tag="blk")
                    for d in range(4):
                        nc.tensor.matmul(
                            ps,
                            lhsT=wv[d][:, u * P:(u + 1) * P],
                            rhs=xTg[g][d],
                            start=(d == 0), stop=(d == 3),
                        )
                    if u % 2 == 0:
                        nc.scalar.copy(kT[u][:, g * 512:(g + 1) * 512], ps)
                    else:
                        nc.vector.tensor_copy(kT[u][:, g * 512:(g + 1) * 512], ps)

            def emit_vg(g):
                for sc in range(4 * g, 4 * g + 4):
                    ps = psA.tile([P, 512], f32, tag="blk")
                    for d in range(4):
                        nc.tensor.matmul(
                            ps,
                            lhsT=xTg[g][d][:, (sc % 4) * P:(sc % 4 + 1) * P],
                            rhs=wk[d],
                            start=(d == 0), stop=(d == 3),
                        )
                    if sc % 2 == 0:
                        nc.scalar.copy(v_sb[sc], ps)
                    else:
                        nc.vector.tensor_copy(v_sb[sc], ps)

            # ---- streaming schedule: projections per xT chunk, attention
            # slots as their kT/v dependencies complete ----
            emit_ktg(0)
            for u in (1, 2, 3):
                emit_qt(u, 0)  # wq tail arrives during kTg0
            emit_vg(0)
            emit_slot(0)
            emit_slot(1)
            emit_ktg(1)
            emit_vg(1)
            for u in range(4):
                emit_qt(u, 1)  # only needed from slot 4 on; qx1 arrives late
            emit_slot(2)
            emit_slot(3)
            emit_ktg(2)
            emit_vg(2)
            emit_slot(4)
            emit_slot(5)
            emit_ktg(3)
            emit_vg(3)

            # ---- mean-of-v (for the fully-masked global row 0) ----
            xs16 = []
            for d in range(4):
                xs = small.tile([P, 4], f32, tag="xs")
                for g in range(4):
                    nc.vector.reduce_sum(xs[:, g:g + 1], xTg[g][d],
                                         axis=mybir.AxisListType.X)
                xst = small.tile([P, 1], f32, tag="xst")
                nc.vector.reduce_sum(xst, xs, axis=mybir.AxisListType.X)
                x16 = small.tile([P, 1], bf16, tag="xs16")
                nc.vector.tensor_copy(x16, xst)
                xs16.append(x16)
            vm_ps = psA.tile([1, 512], f32, tag="blk")
            for d in range(4):
                nc.tensor.matmul(vm_ps, lhsT=xs16[d], rhs=wk[d],
                                 start=(d == 0), stop=(d == 3))
            vm_sb = cst.tile([1, 512], f32, tag="vm_sb")
            # vm_sb = sum_s v[s, :] * rscale  (rscale = 1/S for role 0, else 0)
            nc.vector.tensor_scalar_mul(vm_sb, vm_ps, rsel[0:1, 1:2])
            # row 0 of role 0 = mean(v): ctx*rsel + sum(v)*rscale
            nc.vector.tensor_scalar_mul(ctx0_sb[0:1, :], ctx0_sb[0:1, :],
                                        rsel[0:1, 0:1])
            nc.vector.tensor_add(ctx0_sb[0:1, :], ctx0_sb[0:1, :], vm_sb)
            nc.sync.dma_start(out=out_d[0:P, :], in_=ctx0_sb)

            emit_slot(6)
            emit_slot(7)

    nc.compile()
    _nc_cache = nc
    return nc


def host_inputs(query, Wq, Wv, Wk):
    """Build per-core input maps. query [B,S,D] f32; W* [D,U] f32."""
    wq16 = Wq.astype(BF16)
    wv16 = Wv.astype(BF16)
    wk16 = Wk.astype(BF16)

    i = np.arange(P)[:, None]
    j = np.arange(P)[None, :]
    tri = (i < j).astype(np.float32)  # keep kk < q within the diagonal chunk
    masks = {
        0: np.concatenate([tri, np.zeros((P, P), np.float32)], 1).astype(BF16),
        1: np.concatenate([np.ones((P, P), np.float32), tri], 1).astype(BF16),
    }

    in_maps = []
    for core in range(8):
        b, r = core // 2, core % 2
        xTb = np.ascontiguousarray(query[b].T).astype(BF16)       # [D, S]
        cols = np.concatenate(
            [np.arange(P * (2 * s + r), P * (2 * s + r) + P)
             for s in range(NSLOT)]
        )
        qx = np.ascontiguousarray(xTb[:, cols])                    # [D, 1024]
        rsel = np.array([[0.0, 1.0 / S]] if r == 0 else [[1.0, 0.0]], np.float32)
        sume = np.zeros((P, 1), np.float32)
        if r == 0:
            sume[0, 0] = 1.0  # avoid 1/0 on the fully-masked row
        in_maps.append({
            "xT": xTb, "qx": qx,
            "wq": wq16, "wv": wv16, "wk": wk16,
            "msk": masks[r], "rsel": rsel, "sume": sume,
        })
    return in_maps


def assemble_output(results):
    """results: list of 8 dicts with 'out' [1024, 512] bf16."""
    out = np.zeros((B, S, U), np.float32)
    for core in range(8):
        b, r = core // 2, core % 2
        o = np.asarray(results[core]["out"]).astype(np.float32)
        for s in range(NSLOT):
            t = 2 * s + r
            out[b, P * t:P * (t + 1), :] = o[P * s:P * (s + 1), :]
    return out


def run(query, Wq, Wv, Wk, **kwargs):
    """Build, compile, and execute on all 8 cores. Returns (output, results)."""
    nc = build_nc()
    in_maps = host_inputs(
        np.asarray(query, np.float32), np.asarray(Wq, np.float32),
        np.asarray(Wv, np.float32), np.asarray(Wk, np.float32),
    )
    res = bass_utils.run_bass_kernel_spmd(nc, in_maps, list(range(8)), **kwargs)
    return assemble_output(res.results), res


def kernel(query, Wq, Wv, Wk):
    out, _ = run(query, Wq, Wv, Wk)
    return out


if __name__ == "__main__":
    rng = np.random.default_rng(0)
    q = rng.standard_normal((B, S, D), dtype=np.float32)
    scale = np.sqrt(2.0 / (D + U)).astype(np.float32)
    Wq = rng.standard_normal((D, U), dtype=np.float32) * scale
    Wv = rng.standard_normal((D, U), dtype=np.float32) * scale
    Wk = rng.standard_normal((D, U), dtype=np.float32) * scale
    out = kernel(q, Wq, Wv, Wk)
    print(out.shape, out.dtype, np.abs(out).mean())


# revision 29
# speedup vs baseline: 1.0275x; 1.0066x over previous
"""Trainium2 Bass kernel for nn_Attention_14190571946482.

Single-head causal self-attention with the TF band_part mask quirk:
  q = x @ Wq ; k = x @ Wv ; v = x @ Wk        (naming quirk of the original)
  score = q @ k^T / sqrt(D) + NEG*(j >= i)    (diagonal masked too!)
  out   = softmax(score) @ v
Row 0 is fully masked -> its softmax is exactly uniform over all S
positions, i.e. out[0] = mean_s v[s]; computed via a mean-of-x matmul.

Sharding: 8 cores = 4 batches x 2 roles. Role r owns q-tiles {2i+r}
(128 rows each). Slot i of the SPMD program processes 2(i+1) causal
kk-chunks of 128; role 0's tile 2i needs 2i+1 chunks (one wasted, its
mask is all-zero), role 1's tile 2i+1 needs exactly 2i+2. All
role-dependent structure (qx column gather, chunk masks, row-0 blend
selectors) is carried in input data; the program is identical (SPMD).

Score is computed TRANSPOSED, scoreT[kk, q] = sum_u kT[u,kk]*qT[u,q],
so exp(scoreT) in SBUF is directly the lhsT of the context matmul
(ctx[q,u] = sum_kk attnT[kk,q]*v[kk,u]) -- no PE transposes. Row sums
come from an N=1 matmul with a ones vector accumulated per slot.
Projections (kT g-blocks, v chunks) are interleaved with attention
slots in xT-chunk DMA arrival order so the PE streams without gaps.
"""

import sys

sys.path.insert(0, "/opt/trn_rl_repo")

import numpy as np
import ml_dtypes

import concourse.bass as bass
import concourse.bacc as bacc
import concourse.mybir as mybir
from concourse.tile import TileContext
from concourse import bass_utils

BF16 = ml_dtypes.bfloat16

B, S, D, U = 4, 2048, 512, 512
P = 128
NSLOT = 8
SCALE = 1.0 / float(np.sqrt(np.float32(D)))

_nc_cache = None


def build_nc():
    global _nc_cache
    if _nc_cache is not None:
        return _nc_cache

    f32 = mybir.dt.float32
    bf16 = mybir.dt.bfloat16

    nc = bacc.Bacc()
    xT_d = nc.declare_dram_parameter("xT", [D, S], bf16, isOutput=False)
    qx_d = nc.declare_dram_parameter("qx", [D, NSLOT * P], bf16, isOutput=False)
    wq_d = nc.declare_dram_parameter("wq", [D, U], bf16, isOutput=False)
    wv_d = nc.declare_dram_parameter("wv", [D, U], bf16, isOutput=False)
    wk_d = nc.declare_dram_parameter("wk", [D, U], bf16, isOutput=False)
    mk_d = nc.declare_dram_parameter("msk", [P, 2 * P], bf16, isOutput=False)
    rs_d = nc.declare_dram_parameter("rsel", [1, 2], f32, isOutput=False)
    se_d = nc.declare_dram_parameter("sume", [P, 1], f32, isOutput=False)
    out_d = nc.declare_dram_parameter("out", [NSLOT * P, U], bf16, isOutput=True)

    with TileContext(nc) as tc:
        with (
            tc.tile_pool(name="cst", bufs=1) as cst,
            tc.tile_pool(name="att", bufs=6) as att,
            tc.tile_pool(name="work", bufs=4) as work,
            tc.tile_pool(name="outp", bufs=2) as outp,
            tc.tile_pool(name="small", bufs=8) as small,
            tc.tile_pool(name="psA", bufs=2, space="PSUM") as psA,
            tc.tile_pool(name="psS", bufs=1, space="PSUM") as psS,
            tc.tile_pool(name="psC", bufs=2, space="PSUM") as psC,
            tc.tile_pool(name="psR", bufs=1, space="PSUM") as psR,
        ):
            # ---- input DMA issues (sync/gpsimd/scalar queues; data needed
            # earliest first: wq+qx unblock qT, then wv+xT0 for kT g0) ----
            # Each issuing engine owns one serial DMA descriptor ring; rings
            # share HBM bandwidth, so per-ring issue order = arrival priority.
            # DMA-write dependencies are tracked at TILE granularity: every
            # chunk that must unblock compute separately gets its OWN tile.
            # constants first on gpsimd (warmup matmuls depend on wu)
            wu = cst.tile([P, 512], bf16, tag="wu")
            nc.vector.memset(wu, 0.0)
            ones = cst.tile([P, 1], bf16, tag="ones")
            nc.vector.memset(ones, 1.0)

            wq_r = wq_d.rearrange("(d p) u -> p d u", p=P)
            qx_r = qx_d.rearrange("(d p) s -> p d s", p=P)
            wq0_t = cst.tile([P, 4, P], bf16, tag="wq0")
            nc.sync.dma_start(out=wq0_t, in_=wq_r[:, :, 0:P])
            qx0_t = cst.tile([P, 4, 512], bf16, tag="qx0")
            nc.sync.dma_start(out=qx0_t, in_=qx_r[:, :, 0:512])
            wq1_t = cst.tile([P, 4, U - P], bf16, tag="wq1")
            nc.sync.dma_start(out=wq1_t, in_=wq_r[:, :, P:U])
            qx1_t = cst.tile([P, 4, 512], bf16, tag="qx1")
            nc.sync.dma_start(out=qx1_t, in_=qx_r[:, :, 512:1024])

            xT_r = xT_d.rearrange("(d p) s -> p d s", p=P)
            xTg_t = []
            for g in range(4):
                t = cst.tile([P, 4, 512], bf16, tag=f"xTg{g}", name=f"xTg{g}")
                nc.scalar.dma_start(out=t, in_=xT_r[:, :, g * 512:(g + 1) * 512])
                xTg_t.append(t)

            wv_t = cst.tile([P, 4, U], bf16, tag="wv")
            nc.gpsimd.dma_start(out=wv_t, in_=wv_d.rearrange("(d p) u -> p d u", p=P))
            wk_t = cst.tile([P, 4, U], bf16, tag="wk")
            nc.gpsimd.dma_start(out=wk_t, in_=wk_d.rearrange("(d p) u -> p d u", p=P))
            msk_t = cst.tile([P, 2 * P], bf16, tag="msk")
            nc.gpsimd.dma_start(out=msk_t, in_=mk_d[:, :])
            rsel = cst.tile([1, 2], f32, tag="rsel")
            nc.gpsimd.dma_start(out=rsel, in_=rs_d[:, :])
            sume = cst.tile([P, 1], f32, tag="sume")
            nc.gpsimd.dma_start(out=sume, in_=se_d[:, :])

            def wq(d, lo, hi):  # wq columns [lo:hi] of d-slab, split tiles
                if hi <= P:
                    return wq0_t[:, d, lo:hi]
                return wq1_t[:, d, lo - P:hi - P]

            qx = [[qx0_t[:, d, :] for d in range(4)],
                  [qx1_t[:, d, :] for d in range(4)]]
            xTg = [[xTg_t[g][:, d, :] for d in range(4)] for g in range(4)]
            wv = [wv_t[:, d, :] for d in range(4)]
            wk = [wk_t[:, d, :] for d in range(4)]
            mask0 = msk_t[:, 0:P]
            mask1 = msk_t[:, P:2 * P]

            # ---- PE warm-up / fillers: dummy matmuls keep the PE busy (and
            # the HAM clock ramping) while input DMAs are still in flight.
            # Any PE idle gap resets the ramp, costing ~2x on what follows.
            # Fillers must be dependency-free: during the ramp psC is not yet
            # used by any slot, so its pool rotation creates no cross-engine
            # WARs (psA would chain fillers onto the DMA-gated qT copies).
            def filler(n, pool):
                for _ in range(n):
                    wups = pool.tile([P, 512], f32,
                                     tag="blk" if pool is psA else "ctx")
                    nc.tensor.matmul(wups[:, 0:256], lhsT=wu[:, :P],
                                     rhs=wu[:, 0:256], start=True, stop=True)

            filler(28, psA)

            # ---- persistent PSUM tiles for scoreT chunks: 3 banks x 4
            # quarters, chunk gc -> bank gc%3, quarter (gc//3)%4. With the
            # lead-2 pipeline (ctx_c emitted after sc_{c+2}) the PE never
            # writes a bank while scalar exp-reads another quarter of it:
            # exp_c overlaps only sc_{c+1}/sc_{c+2} (different banks), and
            # sc_{c+3} (same bank) queues after ctx_c which waits on exp_c.
            # (PE-write + engine-read of one PSUM bank is fatal on HW.)
            scT = [psS.tile([P, 512], f32, tag=f"sc{i}", name=f"sc{i}")
                   for i in range(3)]
            rsum = psR.tile([P, 2], f32, tag="rsum")

            # ---- qT [u, 8*128] from gathered qx ----
            qT = [cst.tile([P, NSLOT * P], bf16, tag=f"qT{u}", name=f"qT{u}")
                  for u in range(4)]
            def emit_qt(u, h):
                ps = psA.tile([P, 512], f32, tag="blk")
                for d in range(4):
                    nc.tensor.matmul(
                        ps,
                        lhsT=wq(d, u * P, (u + 1) * P),
                        rhs=qx[h][d],
                        start=(d == 0), stop=(d == 3),
                    )
                nc.scalar.copy(qT[u][:, h * 512:(h + 1) * 512], ps)

            emit_qt(0, 0)
            filler(6, psC)  # cover the wq-tail / xTg0 arrival stagger

            kT = [cst.tile([P, S], bf16, tag=f"kT{u}", name=f"kT{u}")
                  for u in range(4)]
            v_sb = [cst.tile([P, U], bf16, tag=f"v{sc}", name=f"v{sc}")
                    for sc in range(16)]

            gc = 0  # global score-chunk counter for psS rotation

            def emit_score(cols, width, c):
                """scoreT chunk c for qT columns [cols, cols+width)."""
                nonlocal gc
                t_ps = scT[gc % 3]
                half = (gc // 3) % 2
                ps = t_ps[:, half * 256:half * 256 + width]
                gc += 1
                for d in range(4):
                    nc.tensor.matmul(
                        ps,
                        lhsT=kT[d][:, c * P:(c + 1) * P],
                        rhs=qT[d][:, cols:cols + width],
                        start=(d == 0), stop=(d == 3),
                    )
                return ps

            ctx0_sb = cst.tile([P, 512], bf16, tag="ctx0")

            def emit_pair(j):
                """Attention for slots (2j, 2j+1) together: score chunks span
                both slots' q columns (256 wide) while both need chunk c,
                halving LDWEIGHTS pressure; slot 2j+1's two extra chunks run
                single-width. Row sums accumulate in rsum cols 0/1; both
                epilogues run at pair end (PSUM bank read only after the PE
                closed both accumulation groups)."""
                s_lo, s_hi = 2 * j, 2 * j + 1
                n_lo, n_hi = 4 * j + 2, 4 * j + 4
                ctxL = psC.tile([P, 512], f32, tag="ctx")
                ctxH = psC.tile([P, 512], f32, tag="ctx")
                pending = []  # (attnT tile, chunk, paired)

                def pop_ctx():
                    # NB: a matmul start=True clears has_written for the WHOLE
                    # psum bank, so only the pair's very first rowsum matmul
                    # may carry it; the hi column initializes via the
                    # overwrite-where-clear semantics of start=False.
                    a, c, paired = pending.pop(0)
                    if paired:
                        nc.tensor.matmul(ctxL, lhsT=a[:, 0:P], rhs=v_sb[c],
                                         start=(c == 0), stop=(c == n_lo - 1))
                        nc.tensor.matmul(rsum[:, 0:1], lhsT=a[:, 0:P],
                                         rhs=ones,
                                         start=(c == 0), stop=(c == n_lo - 1),
                                         skip_group_check=True)
                        hi = a[:, P:2 * P]
                    else:
                        hi = a[:, 0:P]
                    nc.tensor.matmul(ctxH, lhsT=hi, rhs=v_sb[c],
                                     start=(c == 0), stop=(c == n_hi - 1))
                    nc.tensor.matmul(rsum[:, 1:2], lhsT=hi, rhs=ones,
                                     start=False, stop=(c == n_hi - 1),
                                     skip_group_check=True)

                for c in range(n_hi):
                    paired = c < n_lo
                    width = 256 if paired else P
                    cols = s_lo * P if paired else s_hi * P
                    ps = emit_score(cols, width, c)
                    a = att.tile([P, 256], bf16, tag="attnT")
                    nc.scalar.activation(a[:, 0:width], ps,
                                         mybir.ActivationFunctionType.Exp,
                                         scale=SCALE)
                    if paired and c >= n_lo - 2:
                        m = mask0 if c == n_lo - 2 else mask1
                        nc.vector.tensor_mul(a[:, 0:P], a[:, 0:P], m)
                    if not paired:
                        m = mask0 if c == n_hi - 2 else mask1
                        nc.vector.tensor_mul(a[:, 0:P], a[:, 0:P], m)
                    if len(pending) >= 2:
                        pop_ctx()
                    pending.append((a, c, paired))
                while pending:
                    pop_ctx()
                # epilogues (vector only; scalar stays free for exps)
                rcpL = small.tile([P, 1], f32, tag="rcp")
                if j == 0:
                    rs2 = small.tile([P, 1], f32, tag="rs2")
                    nc.vector.tensor_add(rs2, rsum[:, 0:1], sume)
                    nc.vector.reciprocal(rcpL, rs2)
                else:
                    nc.vector.reciprocal(rcpL, rsum[:, 0:1])
                rcpH = small.tile([P, 1], f32, tag="rcp")
                nc.vector.reciprocal(rcpH, rsum[:, 1:2])
                if j == 0:
                    nc.vector.tensor_scalar_mul(ctx0_sb, ctxL, rcpL)
                    # blend + store deferred until mean-v is ready
                else:
                    ctx16 = outp.tile([P, 512], bf16, tag="ctxs")
                    nc.vector.tensor_scalar_mul(ctx16, ctxL, rcpL)
                    nc.sync.dma_start(out=out_d[s_lo * P:(s_lo + 1) * P, :],
                                      in_=ctx16)
                ctx16h = outp.tile([P, 512], bf16, tag="ctxs")
                nc.vector.tensor_scalar_mul(ctx16h, ctxH, rcpH)
                nc.sync.dma_start(out=out_d[s_hi * P:(s_hi + 1) * P, :],
                                  in_=ctx16h)

            def emit_ktg(g):
                for u in range(4):
                    ps = psA.tile([P, 512], f32, tag="blk")
                    for d in range(4):
                        nc.tensor.matmul(
                            ps,
                            lhsT=wv[d][:, u * P:(u + 1) * P],
                            rhs=xTg[g][d],
                            start=(d == 0), stop=(d == 3),
                        )
                    if u % 2 == 0:
                        nc.scalar.copy(kT[u][:, g * 512:(g + 1) * 512], ps)
                    else:
                        nc.vector.tensor_copy(kT[u][:, g * 512:(g + 1) * 512], ps)

            def emit_vg(g):
                for sc in range(4 * g, 4 * g + 4):
                    ps = psA.tile([P, 512], f32, tag="blk")
                    for d in range(4):
                        nc.tensor.matmul(
                            ps,
                            lhsT=xTg[g][d][:, (sc % 4) * P:(sc % 4 + 1) * P],
                            rhs=wk[d],
                            start=(d == 0), stop=(d == 3),
                        )
                    if sc % 2 == 0:
                        nc.scalar.copy(v_sb[sc], ps)
                    else:
                        nc.vector.tensor_copy(v_sb[sc], ps)

            # ---- streaming schedule: projections per xT chunk, attention
            # slots as their kT/v dependencies complete ----
            emit_ktg(0)
            for u in (1, 2, 3):
                emit_qt(u, 0)  # wq tail arrives during kTg0
            emit_vg(0)
            emit_pair(0)
            emit_ktg(1)
            emit_vg(1)
            for u in range(4):
                emit_qt(u, 1)  # only needed from slot 4 on; qx1 arrives late
            emit_pair(1)
            emit_ktg(2)
            emit_vg(2)
            emit_pair(2)
            emit_ktg(3)
            emit_vg(3)

            # ---- mean-of-v (for the fully-masked global row 0) ----
            xs16 = []
            for d in range(4):
                xs = small.tile([P, 4], f32, tag="xs")
                for g in range(4):
                    nc.vector.reduce_sum(xs[:, g:g + 1], xTg[g][d],
                                         axis=mybir.AxisListType.X)
                xst = small.tile([P, 1], f32, tag="xst")
                nc.vector.reduce_sum(xst, xs, axis=mybir.AxisListType.X)
                x16 = small.tile([P, 1], bf16, tag="xs16")
                nc.vector.tensor_copy(x16, xst)
                xs16.append(x16)
            vm_ps = psA.tile([1, 512], f32, tag="blk")
            for d in range(4):
                nc.tensor.matmul(vm_ps, lhsT=xs16[d], rhs=wk[d],
                                 start=(d == 0), stop=(d == 3))
            vm_sb = cst.tile([1, 512], f32, tag="vm_sb")
            # vm_sb = sum_s v[s, :] * rscale  (rscale = 1/S for role 0, else 0)
            nc.vector.tensor_scalar_mul(vm_sb, vm_ps, rsel[0:1, 1:2])
            # row 0 of role 0 = mean(v): ctx*rsel + sum(v)*rscale
            nc.vector.tensor_scalar_mul(ctx0_sb[0:1, :], ctx0_sb[0:1, :],
                                        rsel[0:1, 0:1])
            nc.vector.tensor_add(ctx0_sb[0:1, :], ctx0_sb[0:1, :], vm_sb)
            nc.sync.dma_start(out=out_d[0:P, :], in_=ctx0_sb)

            emit_pair(3)

    nc.compile()
    _nc_cache = nc
    return nc


def host_inputs(query, Wq, Wv, Wk):
    """Build per-core input maps. query [B,S,D] f32; W* [D,U] f32."""
    wq16 = Wq.astype(BF16)
    wv16 = Wv.astype(BF16)
    wk16 = Wk.astype(BF16)

    i = np.arange(P)[:, None]
    j = np.arange(P)[None, :]
    tri = (i < j).astype(np.float32)  # keep kk < q within the diagonal chunk
    masks = {
        0: np.concatenate([tri, np.zeros((P, P), np.float32)], 1).astype(BF16),
        1: np.concatenate([np.ones((P, P), np.float32), tri], 1).astype(BF16),
    }

    in_maps = []
    for core in range(8):
        b, r = core // 2, core % 2
        xTb = np.ascontiguousarray(query[b].T).astype(BF16)       # [D, S]
        cols = np.concatenate(
            [np.arange(P * (2 * s + r), P * (2 * s + r) + P)
             for s in range(NSLOT)]
        )
        qx = np.ascontiguousarray(xTb[:, cols])                    # [D, 1024]
        rsel = np.array([[0.0, 1.0 / S]] if r == 0 else [[1.0, 0.0]], np.float32)
        sume = np.zeros((P, 1), np.float32)
        if r == 0:
            sume[0, 0] = 1.0  # avoid 1/0 on the fully-masked row
        in_maps.append({
            "xT": xTb, "qx": qx,
            "wq": wq16, "wv": wv16, "wk": wk16,
            "msk": masks[r], "rsel": rsel, "sume": sume,
        })
    return in_maps


def assemble_output(results):
    """results: list of 8 dicts with 'out' [1024, 512] bf16."""
    out = np.zeros((B, S, U), np.float32)
    for core in range(8):
        b, r = core // 2, core % 2
        o = np.asarray(results[core]["out"]).astype(np.float32)
        for s in range(NSLOT):
            t = 2 * s + r
            out[b, P * t:P * (t + 1), :] = o[P * s:P * (s + 1), :]
    return out


def run(query, Wq, Wv, Wk, **kwargs):
    """Build, compile, and execute on all 8 cores. Returns (output, results)."""
    nc = build_nc()
    in_maps = host_inputs(
        np.asarray(query, np.float32), np.asarray(Wq, np.float32),
        np.asarray(Wv, np.float32), np.asarray(Wk, np.float32),
    )
    res = bass_utils.run_bass_kernel_spmd(nc, in_maps, list(range(8)), **kwargs)
    return assemble_output(res.results), res


def kernel(query, Wq, Wv, Wk):
    out, _ = run(query, Wq, Wv, Wk)
    return out


if __name__ == "__main__":
    rng = np.random.default_rng(0)
    q = rng.standard_normal((B, S, D), dtype=np.float32)
    scale = np.sqrt(2.0 / (D + U)).astype(np.float32)
    Wq = rng.standard_normal((D, U), dtype=np.float32) * scale
    Wv = rng.standard_normal((D, U), dtype=np.float32) * scale
    Wk = rng.standard_normal((D, U), dtype=np.float32) * scale
    out = kernel(q, Wq, Wv, Wk)
    print(out.shape, out.dtype, np.abs(out).mean())
